# revision 4
# baseline (speedup 1.0000x reference)
"""Trainium2 Bass kernel for nn_Net_1975684956439 (scatter_memory).

Computation (reference):
  e_pa  = sum(coeffs * weight1) / num_atoms + bias1                  # [1]
  f     = -sum(coeffs_derivs * weight1, axis=3)                      # [1, 3, P]
  out_f = segment_sum(f[0].T, neigh_atom_index, num_atoms).T[None]   # [1, 3, N]

Strategy: data-parallel over the pair axis P=131072 across 8 NeuronCores
(16384 pairs/core, 4 blocks of 4096 pairs).  Per core and block:
  - 3 DMAs (one per force component) land the block's coeffs_derivs as
    one [128, 6144] fp32 tile (partition p holds 32 consecutive pairs).
  - ACT casts fp32 -> bf16 (one op per block).
  - DVE multiplies by -weight1 broadcast along free (one op, 2x mode).
  - GPSIMD halves 64 -> 32 per pair, DVE grouped-reduces 32 -> 1 giving
    per-pair forces fb[p, c*32+j].
  - Scatter via one-hot matmul with atom n = 128*q + r:
      DVE builds r-one-hot mt[p, r*32+j] = (iota == neigh%128) and
      q-one-hot qm[p, q*32+j]; GPSIMD forms g[p, (j,q,c)] = qm * fb;
      TensorE accumulates psum[r, q*3+c] += onehot_r.T @ g.
  - e_pa partial: multiply+reduce of this core's coeffs slice.
Host sums the 8 per-core partials ([128,48] each) and rearranges.
"""

import numpy as np
import ml_dtypes

N_CORES = 8
NUM_ATOMS = 2048
ND = 64            # descriptors
P_TOT = 131072     # pairs
PLOC = P_TOT // N_CORES   # 16384 pairs per core
PPP = 32           # pairs per partition per block (= matmul groups)
NBLK = 4           # blocks per core: 4 * 128 * 32 = 16384
NJ = PPP
FREE = PPP * ND    # 2048 free elements per (component, block) slice
BFREE = 3 * FREE   # 6144 per block tile

_BF16 = ml_dtypes.bfloat16

_CACHE = {}


def _build_nc():
    import concourse.bacc as bacc
    import concourse.tile as tile
    from concourse import mybir

    BF = mybir.dt.bfloat16
    F32 = mybir.dt.float32
    OP = mybir.AluOpType
    AX = mybir.AxisListType.X

    nc = bacc.Bacc("TRN2", target_bir_lowering=False, debug=False,
                   num_devices=N_CORES)

    # cd[b*3 + c] = block b, component c slice [128, 2048]
    cd = nc.dram_tensor("cd", [NBLK * 3, 128, FREE], F32, kind="ExternalInput")
    ridx = nc.dram_tensor("ridx", [128, NBLK * NJ], BF, kind="ExternalInput")
    qidx = nc.dram_tensor("qidx", [128, NBLK * NJ], BF, kind="ExternalInput")
    iorep = nc.dram_tensor("iorep", [128, 128 * NJ], BF, kind="ExternalInput")
    io16r = nc.dram_tensor("io16r", [128, 16 * NJ], BF, kind="ExternalInput")
    wneg = nc.dram_tensor("wneg", [128, ND], BF, kind="ExternalInput")
    wpos = nc.dram_tensor("wpos", [128, ND], F32, kind="ExternalInput")
    coef = nc.dram_tensor("coef", [128, 2 * ND], F32, kind="ExternalInput")
    pf = nc.dram_tensor("pf", [128, 48], F32, kind="ExternalOutput")
    pe = nc.dram_tensor("pe", [128, 2], F32, kind="ExternalOutput")

    with tile.TileContext(nc) as tc:
        with nc.allow_low_precision("bf16 pipeline validated against fp32 reference"), \
             tc.tile_pool(name="singles", bufs=1) as singles, \
             tc.tile_pool(name="cdf", bufs=3) as cdf_pool, \
             tc.tile_pool(name="cdb", bufs=2) as cdb_pool, \
             tc.tile_pool(name="tmp", bufs=2) as tmp_pool, \
             tc.tile_pool(name="hh", bufs=2) as h_pool, \
             tc.tile_pool(name="ff", bufs=2) as f_pool, \
             tc.tile_pool(name="qg", bufs=3) as qg_pool, \
             tc.tile_pool(name="mm", bufs=3) as m_pool, \
             tc.tile_pool(name="psum", bufs=1, space="PSUM") as psum_pool:

            # Issue the first blocks' big DMAs before anything else so the
            # HBM pipe starts right at the preamble barrier.
            cdf_tiles = {}
            for b in range(2):
                cdf = cdf_pool.tile([128, BFREE], F32)
                cdf_tiles[b] = cdf
                for c in range(3):
                    nc.sync.dma_start(out=cdf[:, c * FREE:(c + 1) * FREE],
                                      in_=cd[b * 3 + c, :, :])

            s_ridx = singles.tile([128, NBLK * NJ], BF)
            nc.sync.dma_start(out=s_ridx, in_=ridx[:, :])
            s_qidx = singles.tile([128, NBLK * NJ], BF)
            nc.sync.dma_start(out=s_qidx, in_=qidx[:, :])
            s_iorep = singles.tile([128, 128 * NJ], BF)
            nc.sync.dma_start(out=s_iorep, in_=iorep[:, :])
            s_io16r = singles.tile([128, 16 * NJ], BF)
            nc.sync.dma_start(out=s_io16r, in_=io16r[:, :])
            s_wneg = singles.tile([128, ND], BF)
            nc.sync.dma_start(out=s_wneg, in_=wneg[:, :])
            s_wpos = singles.tile([128, ND], F32)
            nc.sync.dma_start(out=s_wpos, in_=wpos[:, :])
            s_coef = singles.tile([128, 2 * ND], F32)
            nc.sync.dma_start(out=s_coef, in_=coef[:, :])

            pfp = psum_pool.tile([128, 48], F32)

            for b in range(NBLK):
                if b in cdf_tiles:
                    cdf = cdf_tiles[b]
                else:
                    cdf = cdf_pool.tile([128, BFREE], F32)
                    for c in range(3):
                        nc.sync.dma_start(out=cdf[:, c * FREE:(c + 1) * FREE],
                                          in_=cd[b * 3 + c, :, :])

                # one-hot tables for this block (no cd dependency)
                qm = qg_pool.tile([128, 16 * NJ], BF, tag="qm")
                nc.vector.tensor_tensor(
                    out=qm[:].rearrange("p (q j) -> p q j", j=NJ),
                    in0=s_io16r[:].rearrange("p (q j) -> p q j", j=NJ),
                    in1=s_qidx[:, b * NJ:(b + 1) * NJ].unsqueeze(1)
                        .broadcast_to((128, 16, NJ)),
                    op=OP.is_equal)
                mt = m_pool.tile([128, 128 * NJ], BF)
                nc.vector.tensor_tensor(
                    out=mt[:].rearrange("p (r j) -> p r j", j=NJ),
                    in0=s_iorep[:].rearrange("p (r j) -> p r j", j=NJ),
                    in1=s_ridx[:, b * NJ:(b + 1) * NJ].unsqueeze(1)
                        .broadcast_to((128, 128, NJ)),
                    op=OP.is_equal)

                cdb = cdb_pool.tile([128, BFREE], BF)
                nc.scalar.copy(out=cdb, in_=cdf)          # ACT cast
                tmp = tmp_pool.tile([128, BFREE], BF)
                nc.vector.tensor_tensor(                   # DVE: * (-w), 2x
                    out=tmp[:].rearrange("p (m k) -> p m k", k=ND),
                    in0=cdb[:].rearrange("p (m k) -> p m k", k=ND),
                    in1=s_wneg[:].unsqueeze(1).broadcast_to((128, 96, ND)),
                    op=OP.mult)
                t3 = tmp[:].rearrange("p (m k) -> p m k", k=ND)
                h1 = h_pool.tile([128, 96 * 32], BF)
                h1v = h1[:].rearrange("p (m k) -> p m k", k=32)
                nc.gpsimd.tensor_tensor(                   # GPSIMD: 64->32
                    out=h1v, in0=t3[:, :, 0:32], in1=t3[:, :, 32:64],
                    op=OP.add)
                fb = f_pool.tile([128, 96], BF)            # fb[p, c*32+j]
                nc.vector.tensor_reduce(                   # DVE: 32->1
                    out=fb,
                    in_=h1[:].rearrange("p (m k) -> p m k", k=32),
                    axis=AX, op=OP.add)

                # g[p, j*48 + q*3 + c] = qm[p, q*32+j] * fb[p, c*32+j]
                g = qg_pool.tile([128, NJ * 48], BF, tag="g")
                nc.gpsimd.tensor_tensor(
                    out=g[:].rearrange("p (j q c) -> p j q c", q=16, c=3),
                    in0=qm[:].rearrange("p (q j) -> p q j", j=NJ)
                        .transpose([0, 2, 1])
                        .unsqueeze(3).broadcast_to((128, NJ, 16, 3)),
                    in1=fb[:].rearrange("p (c j) -> p c j", j=NJ)
                        .transpose([0, 2, 1])
                        .unsqueeze(2).broadcast_to((128, NJ, 16, 3)),
                    op=OP.mult)

                # scatter: psum[r, q*3+c] += onehot_r.T @ g
                mtv = mt[:].rearrange("p (r j) -> p r j", j=NJ)
                for j in range(NJ):
                    nc.tensor.matmul(
                        pfp,
                        lhsT=mtv[:, :, j],
                        rhs=g[:, j * 48:(j + 1) * 48],
                        start=(b == 0 and j == 0),
                        stop=(b == NBLK - 1 and j == NJ - 1))

            # ---- e_pa partial: sum(coeffs * w) over this core's 256 atoms
            etmp = singles.tile([128, 2 * ND], F32)
            nc.vector.tensor_tensor(
                out=etmp[:].rearrange("p (a k) -> p a k", k=ND),
                in0=s_coef[:].rearrange("p (a k) -> p a k", k=ND),
                in1=s_wpos[:].unsqueeze(1).broadcast_to((128, 2, ND)),
                op=OP.mult)
            pe_sb = singles.tile([128, 2], F32)
            nc.vector.tensor_reduce(
                out=pe_sb, in_=etmp[:].rearrange("p (a k) -> p a k", k=ND),
                axis=AX, op=OP.add)
            nc.sync.dma_start(out=pe[:, :], in_=pe_sb)

            pf_sb = singles.tile([128, 48], F32)
            nc.scalar.copy(out=pf_sb, in_=pfp)
            nc.sync.dma_start(out=pf[:, :], in_=pf_sb)

    nc.compile()
    return nc


def _get_nc():
    if "nc" not in _CACHE:
        _CACHE["nc"] = _build_nc()
    return _CACHE["nc"]


def _host_prep(inputs):
    coeffs = np.asarray(inputs["coeffs"])            # [1, 2048, 64] f32
    cd_full = np.asarray(inputs["coeffs_derivs"])    # [1, 3, P, 64] f32
    nei = np.asarray(inputs["neigh_atom_index"])     # [P] int32
    w = np.asarray(inputs["weight1"])                # [1, 64] f32

    iorep = np.ascontiguousarray(np.tile(
        np.repeat(np.arange(128, dtype=np.float32), NJ).astype(_BF16),
        (128, 1)))
    io16r = np.ascontiguousarray(np.tile(
        np.repeat(np.arange(16, dtype=np.float32), NJ).astype(_BF16),
        (128, 1)))
    wneg = np.ascontiguousarray(np.tile((-w[0]).astype(_BF16), (128, 1)))
    wpos = np.ascontiguousarray(np.tile(w[0].astype(np.float32), (128, 1)))

    atoms_per_core = NUM_ATOMS // N_CORES            # 256
    in_maps = []
    for m in range(N_CORES):
        sl = slice(m * PLOC, (m + 1) * PLOC)
        # [3, 16384, 64] -> [3, b, 128, 2048] -> [b, 3, 128, 2048]
        cdm = np.ascontiguousarray(
            np.asarray(cd_full[0, :, sl, :])
            .reshape(3, NBLK, 128, FREE).transpose(1, 0, 2, 3)
            .reshape(NBLK * 3, 128, FREE))
        nm = nei[sl].reshape(NBLK, 128, NJ)          # [b, part, j]
        r = np.ascontiguousarray(
            (nm % 128).astype(np.float32).astype(_BF16)
            .transpose(1, 0, 2).reshape(128, NBLK * NJ))
        q = np.ascontiguousarray(
            (nm // 128).astype(np.float32).astype(_BF16)
            .transpose(1, 0, 2).reshape(128, NBLK * NJ))
        cf = np.ascontiguousarray(
            coeffs[0, m * atoms_per_core:(m + 1) * atoms_per_core, :]
            .reshape(2, 128, ND).transpose(1, 0, 2).reshape(128, 2 * ND))
        in_maps.append(dict(cd=cdm, ridx=r, qidx=q, iorep=iorep,
                            io16r=io16r, wneg=wneg, wpos=wpos, coef=cf))
    return in_maps


def run(inputs, trace=False, trace_kwargs=None):
    """Run the kernel; returns ((e_pa, out_f), BassKernelResults)."""
    from concourse.bass_utils import run_bass_kernel_spmd

    nc = _get_nc()
    in_maps = _host_prep(inputs)
    res = run_bass_kernel_spmd(
        nc, in_maps, core_ids=list(range(N_CORES)), trace=trace,
        **(trace_kwargs or {}))

    pf = np.zeros((128, 48), np.float64)
    pe_total = 0.0
    for i in range(N_CORES):
        pf += res.results[i]["pf"].astype(np.float64)
        pe_total += float(res.results[i]["pe"].sum(dtype=np.float64))

    out_f = np.ascontiguousarray(
        pf.reshape(128, 16, 3).transpose(2, 1, 0).reshape(1, 3, NUM_ATOMS)
        .astype(np.float32))
    bias1 = np.asarray(inputs["bias1"]).astype(np.float32)
    e_pa = (np.array([pe_total / NUM_ATOMS], np.float32) + bias1).astype(
        np.float32)
    return (e_pa, out_f), res


def kernel(**inputs):
    (e_pa, out_f), _ = run(inputs, trace=False)
    return (e_pa, out_f)


# revision 6
# speedup vs baseline: 1.1912x; 1.1912x over previous
"""Trainium2 Bass kernel for nn_Net_1975684956439 (scatter_memory).

Computation (reference):
  e_pa  = sum(coeffs * weight1) / num_atoms + bias1                  # [1]
  f     = -sum(coeffs_derivs * weight1, axis=3)                      # [1, 3, P]
  out_f = segment_sum(f[0].T, neigh_atom_index, num_atoms).T[None]   # [1, 3, N]

Strategy: data-parallel over the pair axis P=131072 across 8 NeuronCores
(16384 pairs/core, 4 blocks of 4096 pairs).  Per core and block:
  - 3 DMAs (one per force component) land the block's coeffs_derivs as
    one [128, 6144] fp32 tile (partition p holds 32 consecutive pairs).
  - ACT casts fp32 -> bf16 (one op per block).
  - DVE multiplies by -weight1 broadcast along free (one op, 2x mode).
  - GPSIMD halves 64 -> 32 per pair, DVE grouped-reduces 32 -> 1 giving
    per-pair forces fb[p, c*32+j].
  - Scatter via one-hot matmul with atom n = 128*q + r:
      DVE builds r-one-hot mt[p, r*32+j] = (iota == neigh%128) and
      q-one-hot qm[p, q*32+j]; GPSIMD forms g[p, (j,q,c)] = qm * fb;
      TensorE accumulates psum[r, q*3+c] += onehot_r.T @ g.
  - e_pa partial: multiply+reduce of this core's coeffs slice.
Host sums the 8 per-core partials ([128,48] each) and rearranges.
"""

import numpy as np
import ml_dtypes

N_CORES = 8
NUM_ATOMS = 2048
ND = 64            # descriptors
P_TOT = 131072     # pairs
PLOC = P_TOT // N_CORES   # 16384 pairs per core
PPP = 32           # pairs per partition per block (= matmul groups)
NBLK = 4           # blocks per core: 4 * 128 * 32 = 16384
NJ = PPP
FREE = PPP * ND    # 2048 free elements per (component, block) slice
BFREE = 3 * FREE   # 6144 per block tile

_BF16 = ml_dtypes.bfloat16

_CACHE = {}


def _build_nc():
    import concourse.bacc as bacc
    import concourse.tile as tile
    from concourse import mybir

    BF = mybir.dt.bfloat16
    F32 = mybir.dt.float32
    OP = mybir.AluOpType
    AX = mybir.AxisListType.X

    nc = bacc.Bacc("TRN2", target_bir_lowering=False, debug=False,
                   num_devices=N_CORES)

    # cd[b*3 + c] = block b, component c slice [128, 2048]
    cd = nc.dram_tensor("cd", [NBLK * 3, 128, FREE], F32, kind="ExternalInput")
    ridx = nc.dram_tensor("ridx", [128, NBLK * NJ], BF, kind="ExternalInput")
    qidx = nc.dram_tensor("qidx", [128, NBLK * NJ], BF, kind="ExternalInput")
    iorep = nc.dram_tensor("iorep", [128, 128 * NJ], BF, kind="ExternalInput")
    io16r = nc.dram_tensor("io16r", [128, 16 * NJ], BF, kind="ExternalInput")
    wneg = nc.dram_tensor("wneg", [128, ND], BF, kind="ExternalInput")
    wpos = nc.dram_tensor("wpos", [128, ND], F32, kind="ExternalInput")
    coef = nc.dram_tensor("coef", [128, 2 * ND], F32, kind="ExternalInput")
    pf = nc.dram_tensor("pf", [128, 48], F32, kind="ExternalOutput")
    pe = nc.dram_tensor("pe", [128, 2], F32, kind="ExternalOutput")

    with tile.TileContext(nc) as tc:
        with nc.allow_low_precision("bf16 pipeline validated against fp32 reference"), \
             tc.tile_pool(name="singles", bufs=1) as singles, \
             tc.tile_pool(name="cdb", bufs=3) as cdb_pool, \
             tc.tile_pool(name="hh", bufs=3) as h_pool, \
             tc.tile_pool(name="th", bufs=2) as th_pool, \
             tc.tile_pool(name="ff", bufs=2) as f_pool, \
             tc.tile_pool(name="qg", bufs=3) as qg_pool, \
             tc.tile_pool(name="mm", bufs=3) as m_pool, \
             tc.tile_pool(name="psum", bufs=1, space="PSUM") as psum_pool:

            # Issue the first blocks' cast-load DMAs (SWDGE casts fp32->bf16
            # in flight) before anything else so the HBM pipe starts early.
            cdb_tiles = {}
            for b in range(2):
                cdb = cdb_pool.tile([128, BFREE], BF)
                cdb_tiles[b] = cdb
                for c in range(3):
                    nc.gpsimd.dma_start(out=cdb[:, c * FREE:(c + 1) * FREE],
                                        in_=cd[b * 3 + c, :, :])

            s_ridx = singles.tile([128, NBLK * NJ], BF)
            nc.sync.dma_start(out=s_ridx, in_=ridx[:, :])
            s_qidx = singles.tile([128, NBLK * NJ], BF)
            nc.sync.dma_start(out=s_qidx, in_=qidx[:, :])
            s_iorep = singles.tile([128, 128 * NJ], BF)
            nc.sync.dma_start(out=s_iorep, in_=iorep[:, :])
            s_io16r = singles.tile([128, 16 * NJ], BF)
            nc.sync.dma_start(out=s_io16r, in_=io16r[:, :])
            s_wneg = singles.tile([128, ND], BF)
            nc.sync.dma_start(out=s_wneg, in_=wneg[:, :])
            s_wpos = singles.tile([128, ND], F32)
            nc.sync.dma_start(out=s_wpos, in_=wpos[:, :])
            s_coef = singles.tile([128, 2 * ND], F32)
            nc.sync.dma_start(out=s_coef, in_=coef[:, :])

            pfp = psum_pool.tile([128, 48], F32)

            for b in range(NBLK):
                if b in cdb_tiles:
                    cdb = cdb_tiles[b]
                else:
                    cdb = cdb_pool.tile([128, BFREE], BF)
                    for c in range(3):
                        nc.gpsimd.dma_start(
                            out=cdb[:, c * FREE:(c + 1) * FREE],
                            in_=cd[b * 3 + c, :, :])

                # one-hot tables for this block (no cd dependency)
                qm = qg_pool.tile([128, 16 * NJ], BF, tag="qm")
                nc.vector.tensor_tensor(
                    out=qm[:].rearrange("p (q j) -> p q j", j=NJ),
                    in0=s_io16r[:].rearrange("p (q j) -> p q j", j=NJ),
                    in1=s_qidx[:, b * NJ:(b + 1) * NJ].unsqueeze(1)
                        .broadcast_to((128, 16, NJ)),
                    op=OP.is_equal)
                mt = m_pool.tile([128, 128 * NJ], BF)
                nc.vector.tensor_tensor(
                    out=mt[:].rearrange("p (r j) -> p r j", j=NJ),
                    in0=s_iorep[:].rearrange("p (r j) -> p r j", j=NJ),
                    in1=s_ridx[:, b * NJ:(b + 1) * NJ].unsqueeze(1)
                        .broadcast_to((128, 128, NJ)),
                    op=OP.is_equal)

                # matvec: tmp_half[p, m*32+k] = cdb[p, m*64+(off+k)] * -w[off+k]
                cdv = cdb[:].rearrange("p (m h k) -> p m h k", h=2, k=32)
                wv = s_wneg[:].rearrange("p (h k) -> p h k", k=32)
                h1 = h_pool.tile([128, 96 * 32], BF)
                h1v = h1[:].rearrange("p (m k) -> p m k", k=32)
                nc.vector.tensor_tensor(            # lo half, 2x mode
                    out=h1v, in0=cdv[:, :, 0, :],
                    in1=wv[:, 0:1, :].broadcast_to((128, 96, 32)),
                    op=OP.mult)
                th = th_pool.tile([128, 96 * 32], BF)
                thv = th[:].rearrange("p (m k) -> p m k", k=32)
                nc.vector.tensor_tensor(            # hi half, 2x mode
                    out=thv, in0=cdv[:, :, 1, :],
                    in1=wv[:, 1:2, :].broadcast_to((128, 96, 32)),
                    op=OP.mult)
                # fold halves: h1 += th via DMA compute (CCE add).  CCE maxes
                # out at 2048 elements per transfer — slice to stay under.
                for s in range(2):
                    nc.gpsimd.dma_start(out=h1[:, s * 1536:(s + 1) * 1536],
                                        in_=th[:, s * 1536:(s + 1) * 1536],
                                        accum_op=OP.add)
                fb = f_pool.tile([128, 96], BF)            # fb[p, c*32+j]
                nc.vector.tensor_reduce(                   # DVE: 32->1
                    out=fb,
                    in_=h1[:].rearrange("p (m k) -> p m k", k=32),
                    axis=AX, op=OP.add)

                # g[p, j*48 + q*3 + c] = qm[p, q*32+j] * fb[p, c*32+j]
                g = qg_pool.tile([128, NJ * 48], BF, tag="g")
                nc.vector.tensor_tensor(
                    out=g[:].rearrange("p (j q c) -> p j q c", q=16, c=3),
                    in0=qm[:].rearrange("p (q j) -> p q j", j=NJ)
                        .transpose([0, 2, 1])
                        .unsqueeze(3).broadcast_to((128, NJ, 16, 3)),
                    in1=fb[:].rearrange("p (c j) -> p c j", j=NJ)
                        .transpose([0, 2, 1])
                        .unsqueeze(2).broadcast_to((128, NJ, 16, 3)),
                    op=OP.mult)

                # scatter: psum[r, q*3+c] += onehot_r.T @ g
                mtv = mt[:].rearrange("p (r j) -> p r j", j=NJ)
                for j in range(NJ):
                    nc.tensor.matmul(
                        pfp,
                        lhsT=mtv[:, :, j],
                        rhs=g[:, j * 48:(j + 1) * 48],
                        start=(b == 0 and j == 0),
                        stop=(b == NBLK - 1 and j == NJ - 1))

            # ---- e_pa partial: sum(coeffs * w) over this core's 256 atoms
            etmp = singles.tile([128, 2 * ND], F32)
            nc.vector.tensor_tensor(
                out=etmp[:].rearrange("p (a k) -> p a k", k=ND),
                in0=s_coef[:].rearrange("p (a k) -> p a k", k=ND),
                in1=s_wpos[:].unsqueeze(1).broadcast_to((128, 2, ND)),
                op=OP.mult)
            pe_sb = singles.tile([128, 2], F32)
            nc.vector.tensor_reduce(
                out=pe_sb, in_=etmp[:].rearrange("p (a k) -> p a k", k=ND),
                axis=AX, op=OP.add)
            nc.sync.dma_start(out=pe[:, :], in_=pe_sb)

            pf_sb = singles.tile([128, 48], F32)
            nc.vector.tensor_copy(pf_sb, pfp)
            nc.sync.dma_start(out=pf[:, :], in_=pf_sb)

    nc.compile()
    return nc


def _get_nc():
    if "nc" not in _CACHE:
        _CACHE["nc"] = _build_nc()
    return _CACHE["nc"]


def _host_prep(inputs):
    coeffs = np.asarray(inputs["coeffs"])            # [1, 2048, 64] f32
    cd_full = np.asarray(inputs["coeffs_derivs"])    # [1, 3, P, 64] f32
    nei = np.asarray(inputs["neigh_atom_index"])     # [P] int32
    w = np.asarray(inputs["weight1"])                # [1, 64] f32

    iorep = np.ascontiguousarray(np.tile(
        np.repeat(np.arange(128, dtype=np.float32), NJ).astype(_BF16),
        (128, 1)))
    io16r = np.ascontiguousarray(np.tile(
        np.repeat(np.arange(16, dtype=np.float32), NJ).astype(_BF16),
        (128, 1)))
    wneg = np.ascontiguousarray(np.tile((-w[0]).astype(_BF16), (128, 1)))
    wpos = np.ascontiguousarray(np.tile(w[0].astype(np.float32), (128, 1)))

    atoms_per_core = NUM_ATOMS // N_CORES            # 256
    in_maps = []
    for m in range(N_CORES):
        sl = slice(m * PLOC, (m + 1) * PLOC)
        # [3, 16384, 64] -> [3, b, 128, 2048] -> [b, 3, 128, 2048]
        cdm = np.ascontiguousarray(
            np.asarray(cd_full[0, :, sl, :])
            .reshape(3, NBLK, 128, FREE).transpose(1, 0, 2, 3)
            .reshape(NBLK * 3, 128, FREE))
        nm = nei[sl].reshape(NBLK, 128, NJ)          # [b, part, j]
        r = np.ascontiguousarray(
            (nm % 128).astype(np.float32).astype(_BF16)
            .transpose(1, 0, 2).reshape(128, NBLK * NJ))
        q = np.ascontiguousarray(
            (nm // 128).astype(np.float32).astype(_BF16)
            .transpose(1, 0, 2).reshape(128, NBLK * NJ))
        cf = np.ascontiguousarray(
            coeffs[0, m * atoms_per_core:(m + 1) * atoms_per_core, :]
            .reshape(2, 128, ND).transpose(1, 0, 2).reshape(128, 2 * ND))
        in_maps.append(dict(cd=cdm, ridx=r, qidx=q, iorep=iorep,
                            io16r=io16r, wneg=wneg, wpos=wpos, coef=cf))
    return in_maps


def run(inputs, trace=False, trace_kwargs=None):
    """Run the kernel; returns ((e_pa, out_f), BassKernelResults)."""
    from concourse.bass_utils import run_bass_kernel_spmd

    nc = _get_nc()
    in_maps = _host_prep(inputs)
    res = run_bass_kernel_spmd(
        nc, in_maps, core_ids=list(range(N_CORES)), trace=trace,
        **(trace_kwargs or {}))

    pf = np.zeros((128, 48), np.float64)
    pe_total = 0.0
    for i in range(N_CORES):
        pf += res.results[i]["pf"].astype(np.float64)
        pe_total += float(res.results[i]["pe"].sum(dtype=np.float64))

    out_f = np.ascontiguousarray(
        pf.reshape(128, 16, 3).transpose(2, 1, 0).reshape(1, 3, NUM_ATOMS)
        .astype(np.float32))
    bias1 = np.asarray(inputs["bias1"]).astype(np.float32)
    e_pa = (np.array([pe_total / NUM_ATOMS], np.float32) + bias1).astype(
        np.float32)
    return (e_pa, out_f), res


def kernel(**inputs):
    (e_pa, out_f), _ = run(inputs, trace=False)
    return (e_pa, out_f)
